# revision 15
# baseline (speedup 1.0000x reference)
"""Trainium2 Bass kernel for nn_DialogueTransformer.

Reference computation (per batch element b, S=2048 positions, D=1024, H=8 heads
of HD=128):
  x = input_seq + pe
  enc:  q/k/v = x@Wq.T+bq ... ; per-position 8x8 head-mixing softmax attention
        (reshape WITHOUT transpose => heads attend within the same position);
        FFN relu(a@W1.T+b1)@W2.T+b2
  dec:  self-attn on output_seq, cross-attn (q from self-attn out, k/v from
        enc_out), FFN, final fc.

Sharding: pure data-parallel over batch (8 cores, one batch element each).
No collectives.

Device layout: all activations are FEATURE-MAJOR [1024 features, S positions],
split into 8 partition-chunks of 128. q/k/v and attention outputs live in an
INTERLEAVED feature-major layout  [128, S*8]  with column  s*8 + head. This
makes the per-position 8x8 head mixing expressible as dense 128x128 PE matmuls
over 16-position groups:
  scores:  psS[(s,i),(s',j)] = qI_g.T @ kI_g   (diag 8x8 blocks are the real
           scores; off-diag cross-position garbage is masked away)
  softmax: exp on ACT, mask-mul + row-reduce + reciprocal + per-partition
           normalize on DVE
  mix:     transpose W and vI groups on PE, then  out[h,(s,i)] = vPI.T @ WT
The 1/sqrt(HD) score scale is folded into Wq/bq on the host.

Everything is bf16 on the PE with fp32 PSUM accumulation; positions are
processed in NBLK blocks so all block activations fit SBUF; weights stream per
block.
"""

import math
import numpy as np
import ml_dtypes
from contextlib import ExitStack

import concourse.bass as bass
import concourse.tile as tile
from concourse import bacc, mybir
from concourse.bass import ds
from concourse.bass_utils import run_bass_kernel_spmd

BF16 = mybir.dt.bfloat16
F32 = mybir.dt.float32
nbf = ml_dtypes.bfloat16

B, S, D, H, HD = 8, 2048, 1024, 8, 128
NCORES = 8
KC = D // 128          # 8 feature chunks
AF = mybir.ActivationFunctionType
ALU = mybir.AluOpType

# layer index -> (weight name, bias name)
LAYERS = [
    ("enc_wq", "enc_bq"), ("enc_wk", "enc_bk"), ("enc_wv", "enc_bv"),
    ("enc_w1", "enc_b1"), ("enc_w2", "enc_b2"),
    ("dec_s_wq", "dec_s_bq"), ("dec_s_wk", "dec_s_bk"), ("dec_s_wv", "dec_s_bv"),
    ("dec_c_wq", "dec_c_bq"), ("dec_c_wk", "dec_c_bk"), ("dec_c_wv", "dec_c_bv"),
    ("dec_w1", "dec_b1"), ("dec_w2", "dec_b2"), ("fc_w", "fc_b"),
]
LIDX = {name: i for i, (name, _) in enumerate(LAYERS)}


def build_bass(seq=S, nblk=2, repeat=1, variant=""):
    """Build + compile the per-core Bass program. seq = positions per core.

    repeat > 1 wraps the whole body in a device-side For_i loop (for timing:
    marginal wall time per extra iteration = steady-state kernel time).
    variant: timing-only ablations — "noattn" stubs attention with a copy,
    "nostride" replaces strided interleaved rhs reads with contiguous ones
    (results wrong; timing only)."""
    sb_pos = seq // nblk          # positions per block
    nn = sb_pos // 512            # 512-wide position chunks per block
    nch = sb_pos // 64            # attention 4-group chunks per block
    assert sb_pos % 512 == 0 and sb_pos % 64 == 0

    nc = bacc.Bacc("TRN2", target_bir_lowering=False, debug=False)

    xT_d = nc.dram_tensor("xT", [128, KC, seq], BF16, kind="ExternalInput")
    oT_d = nc.dram_tensor("oT", [128, KC, seq], BF16, kind="ExternalInput")
    w_d = [
        nc.dram_tensor(f"w{i}", [128, KC * D], BF16, kind="ExternalInput")
        for i in range(len(LAYERS))
    ]
    bias_d = nc.dram_tensor("biases", [128, len(LAYERS) * KC], F32,
                            kind="ExternalInput")
    mask_d = nc.dram_tensor("mask4", [128, 512], BF16, kind="ExternalInput")
    id_d = nc.dram_tensor("ident", [128, 128], BF16, kind="ExternalInput")
    y_d = nc.dram_tensor("y", [128, KC, seq], F32, kind="ExternalOutput")

    with tile.TileContext(nc) as tc:
        with ExitStack() as ctx:
            const = ctx.enter_context(tc.tile_pool(name="const", bufs=1))
            big = ctx.enter_context(tc.tile_pool(name="big", bufs=1))
            wpool = ctx.enter_context(tc.tile_pool(name="wpool", bufs=2))
            sm = ctx.enter_context(tc.tile_pool(name="sm", bufs=3))
            stg = ctx.enter_context(tc.tile_pool(name="stg", bufs=3))
            ps = ctx.enter_context(tc.tile_pool(name="ps", bufs=4, space="PSUM"))
            pst = ctx.enter_context(tc.tile_pool(name="pst", bufs=2, space="PSUM"))

            bsb = const.tile([128, len(LAYERS) * KC], F32, tag="bias")
            nc.sync.dma_start(bsb[:], bias_d.ap())
            msb = const.tile([128, 512], BF16, tag="mask")
            nc.sync.dma_start(msb[:], mask_d.ap())
            isb = const.tile([128, 128], BF16, tag="ident")
            nc.sync.dma_start(isb[:], id_d.ap())

            def load_weight(li):
                w = wpool.tile([128, KC * D], BF16, tag="w", name=f"w_{li}")
                nc.sync.dma_start(w[:], w_d[li].ap())
                return w

            def load_act(dram, blk, tag):
                t = big.tile([128, KC * sb_pos], BF16, tag=tag, name=tag)
                nc.sync.dma_start(
                    t.rearrange("p (k s) -> p k s", k=KC),
                    dram.ap()[:, :, ds(blk * sb_pos, sb_pos)],
                )
                return t

            def proj(li, rhs_fn, evac_fn):
                """y_chunk[m] = W_li[m,:] @ rhs ; evac_fn(ps, m, n)."""
                w = load_weight(li)
                for m in range(KC):
                    for n in range(nn):
                        pt = ps.tile([128, 512], F32, tag="ps", name=f"ps_{li}_{m}_{n}")
                        for k in range(KC):
                            nc.tensor.matmul(
                                pt[:],
                                w[:, ds(k * D + m * 128, 128)],
                                rhs_fn(k, n),
                                start=(k == 0), stop=(k == KC - 1),
                            )
                        evac_fn(pt, m, n)

            def chunk_rhs(src):
                """rhs from a feature-chunk-major [128, KC*sb_pos] buffer."""
                return lambda k, n: src[:, ds(k * sb_pos + n * 512, 512)]

            def inter_rhs(src_r):
                """rhs from an interleaved [128, sb_pos, 8] buffer."""
                if variant == "nostride":
                    return lambda k, n: src_r.tensor.ap()[:, ds(n * 512, 512)]
                return lambda k, n: src_r[:, ds(n * 512, 512), k]

            def evac_interleaved(dst_r, li):
                """psum -> interleaved dst with bias; alternate ACT/DVE."""
                def f(pt, m, n):
                    bias_ap = bsb[:, ds(li * KC + m, 1)]
                    dst = dst_r[:, ds(n * 512, 512), m]
                    if m % 2 == 0:
                        nc.scalar.activation(dst, pt[:], AF.Identity, bias=bias_ap)
                    else:
                        nc.vector.tensor_scalar(dst, pt[:], bias_ap, None, ALU.add)
                return f

            def evac_chunks(dst, li, relu=False):
                def f(pt, m, n):
                    bias_ap = bsb[:, ds(li * KC + m, 1)]
                    dst_ap = dst[:, ds(m * sb_pos + n * 512, 512)]
                    if m % 2 == 0:
                        nc.scalar.activation(
                            dst_ap, pt[:], AF.Relu if relu else AF.Identity,
                            bias=bias_ap)
                    elif relu:
                        nc.vector.tensor_scalar(
                            dst_ap, pt[:], bias_ap, 0.0, ALU.add, ALU.max)
                    else:
                        nc.vector.tensor_scalar(
                            dst_ap, pt[:], bias_ap, None, ALU.add)
                return f

            def evac_out(blk, li):
                def f(pt, m, n):
                    bias_ap = bsb[:, ds(li * KC + m, 1)]
                    t = stg.tile([128, 512], F32, tag="ystg", name="ystg")
                    if m % 2 == 0:
                        nc.scalar.activation(t[:], pt[:], AF.Identity, bias=bias_ap)
                    else:
                        nc.vector.tensor_scalar(t[:], pt[:], bias_ap, None, ALU.add)
                    nc.sync.dma_start(
                        y_d.ap()[:, m, ds(blk * sb_pos + n * 512, 512)], t[:])
                return f

            def attention(qI, kI, vI, aI):
                if variant == "noattn":
                    for m in range(KC):
                        nc.vector.tensor_copy(
                            aI[:, ds(m * sb_pos, sb_pos)],
                            qI[:, ds(m * sb_pos, sb_pos)])
                    return

                # 3-stage software pipeline over 64-position chunks so the
                # ACT/DVE softmax chain of chunk ch runs under the PE work of
                # chunks ch+1/ch+2 (PE executes its queue in order; without
                # the skew the W-transpose would head-of-line-block PE).
                def s0(ch):
                    base = ch * 512
                    psS = ps.tile([128, 512], F32, tag="psS", name="psS",
                                  bufs=2)
                    for g in range(4):
                        sl = ds(base + g * 128, 128)
                        nc.tensor.matmul(psS[:, ds(g * 128, 128)],
                                         qI[:, sl], kI[:, sl],
                                         start=True, stop=True)
                    return psS

                def s1(ch, psS):
                    E = sm.tile([128, 512], BF16, tag="E", name="E")
                    nc.scalar.activation(E[:], psS[:], AF.Exp)
                    Em = sm.tile([128, 512], BF16, tag="Em", name="Em")
                    nc.vector.tensor_mul(Em[:], E[:], msb[:])
                    den = sm.tile([128, 4], F32, tag="den", name="den")
                    nc.vector.reduce_sum(
                        den[:], Em.rearrange("p (g c) -> p g c", g=4),
                        axis=mybir.AxisListType.X)
                    R = sm.tile([128, 4], F32, tag="R", name="R")
                    nc.vector.reciprocal(R[:], den[:])
                    Wn = sm.tile([128, 512], BF16, tag="Wn", name="Wn")
                    for g in range(4):
                        nc.vector.tensor_scalar_mul(
                            Wn[:, ds(g * 128, 128)], Em[:, ds(g * 128, 128)],
                            R[:, ds(g, 1)])
                    return Wn

                def s2(ch, Wn):
                    base = ch * 512
                    ptv = pst.tile([128, 512], BF16, tag="pst", name="ptv")
                    for g in range(4):
                        nc.tensor.transpose(ptv[:, ds(g * 128, 128)],
                                            vI[:, ds(base + g * 128, 128)], isb[:])
                    vP = sm.tile([128, 512], BF16, tag="vP", name="vP")
                    nc.scalar.copy(vP[:], ptv[:])
                    ptw = pst.tile([128, 512], BF16, tag="pst", name="ptw")
                    for g in range(4):
                        nc.tensor.transpose(ptw[:, ds(g * 128, 128)],
                                            Wn[:, ds(g * 128, 128)], isb[:])
                    WT = sm.tile([128, 512], BF16, tag="WT", name="WT")
                    nc.vector.tensor_copy(WT[:], ptw[:])
                    psO = ps.tile([128, 512], F32, tag="ps", name="psO")
                    for g in range(4):
                        sl = ds(g * 128, 128)
                        nc.tensor.matmul(psO[:, sl], vP[:, sl], WT[:, sl],
                                         start=True, stop=True)
                    nc.scalar.copy(aI[:, ds(base, 512)], psO[:])

                state = {}
                for ch in range(nch + 2):
                    if ch < nch:
                        state[ch] = s0(ch)
                    if 1 <= ch < nch + 1:
                        state[ch - 1] = s1(ch - 1, state[ch - 1])
                    if ch >= 2:
                        s2(ch - 2, state.pop(ch - 2))

            def deinter(aI_r, tag, nm):
                """aI interleaved -> chunk-major copy (strided reads on
                DVE/ACT are ~1x; strided PE rhs reads were ~4x slower)."""
                a_cm = big.tile([128, KC * sb_pos], BF16, tag=tag, name=nm)
                for k in range(KC):
                    dst = a_cm[:, ds(k * sb_pos, sb_pos)]
                    src = aI_r[:, ds(0, sb_pos), k]
                    if k % 2 == 0:
                        nc.scalar.copy(dst, src)
                    else:
                        nc.vector.tensor_copy(dst, src)
                return a_cm

            def qkv_layers(src, li_q, li_k, li_v):
                qI = big.tile([128, sb_pos * 8], BF16, tag="qI", name="qI")
                kI = big.tile([128, sb_pos * 8], BF16, tag="kI", name="kI")
                vI = big.tile([128, sb_pos * 8], BF16, tag="vI", name="vI")
                qI_r = qI.rearrange("p (s e) -> p s e", e=8)
                kI_r = kI.rearrange("p (s e) -> p s e", e=8)
                vI_r = vI.rearrange("p (s e) -> p s e", e=8)
                rhs = chunk_rhs(src)
                proj(li_q, rhs, evac_interleaved(qI_r, li_q))
                proj(li_k, rhs, evac_interleaved(kI_r, li_k))
                proj(li_v, rhs, evac_interleaved(vI_r, li_v))
                return qI, kI, vI

            def body():
                for blk in range(nblk):
                    whole_block(blk)

            def whole_block(blk):
                # ---------------- encoder ----------------
                xc = load_act(xT_d, blk, "xc")
                qI, kI, vI = qkv_layers(
                    xc, LIDX["enc_wq"], LIDX["enc_wk"], LIDX["enc_wv"])
                aI = big.tile([128, sb_pos * 8], BF16, tag="aI", name="aI_enc")
                attention(qI, kI, vI, aI)
                aI_r = aI.rearrange("p (s e) -> p s e", e=8)
                a_cm = deinter(aI_r, "xc", "acm_enc")
                h1 = big.tile([128, KC * sb_pos], BF16, tag="h1", name="h1_enc")
                proj(LIDX["enc_w1"], chunk_rhs(a_cm),
                     evac_chunks(h1, LIDX["enc_w1"], relu=True))
                eo = big.tile([128, KC * sb_pos], BF16, tag="eo", name="eo")
                proj(LIDX["enc_w2"], chunk_rhs(h1),
                     evac_chunks(eo, LIDX["enc_w2"]))

                # ---------------- decoder self-attention ----------------
                oc = load_act(oT_d, blk, "oc")
                qI, kI, vI = qkv_layers(
                    oc, LIDX["dec_s_wq"], LIDX["dec_s_wk"], LIDX["dec_s_wv"])
                aI = big.tile([128, sb_pos * 8], BF16, tag="aI", name="aI_sa")
                attention(qI, kI, vI, aI)
                aI_r = aI.rearrange("p (s e) -> p s e", e=8)
                a_cm = deinter(aI_r, "h1", "acm_sa")

                # ---------------- decoder cross-attention ----------------
                qI = big.tile([128, sb_pos * 8], BF16, tag="qI", name="qI_c")
                qI_r = qI.rearrange("p (s e) -> p s e", e=8)
                proj(LIDX["dec_c_wq"], chunk_rhs(a_cm),
                     evac_interleaved(qI_r, LIDX["dec_c_wq"]))
                kI = big.tile([128, sb_pos * 8], BF16, tag="kI", name="kI_c")
                kI_r = kI.rearrange("p (s e) -> p s e", e=8)
                proj(LIDX["dec_c_wk"], chunk_rhs(eo),
                     evac_interleaved(kI_r, LIDX["dec_c_wk"]))
                vI = big.tile([128, sb_pos * 8], BF16, tag="vI", name="vI_c")
                vI_r = vI.rearrange("p (s e) -> p s e", e=8)
                proj(LIDX["dec_c_wv"], chunk_rhs(eo),
                     evac_interleaved(vI_r, LIDX["dec_c_wv"]))
                aI = big.tile([128, sb_pos * 8], BF16, tag="aI", name="aI_ca")
                attention(qI, kI, vI, aI)
                aI_r = aI.rearrange("p (s e) -> p s e", e=8)
                a_cm = deinter(aI_r, "qI", "acm_ca")

                # ---------------- decoder FFN + fc ----------------
                h1 = big.tile([128, KC * sb_pos], BF16, tag="h1", name="h1_dec")
                proj(LIDX["dec_w1"], chunk_rhs(a_cm),
                     evac_chunks(h1, LIDX["dec_w1"], relu=True))
                d1 = big.tile([128, KC * sb_pos], BF16, tag="xc", name="d1")
                proj(LIDX["dec_w2"], chunk_rhs(h1),
                     evac_chunks(d1, LIDX["dec_w2"]))
                proj(LIDX["fc_w"], chunk_rhs(d1), evac_out(blk, LIDX["fc_w"]))

            if repeat > 1:
                with tc.For_i(0, repeat, 1):
                    body()
            else:
                body()

    nc.compile()
    return nc


def _pack_fm(a):
    """[S, D] fp32 -> feature-major packed [128, KC, S] bf16."""
    s = a.shape[0]
    return np.ascontiguousarray(
        a.T.reshape(KC, 128, s).transpose(1, 0, 2)).astype(nbf)


def _pack_w(w):
    """[d_out, d_in] -> lhsT packed [128, KC*D] bf16 (chunk kc of d_in)."""
    wt = np.ascontiguousarray(w.T)  # [d_in, d_out]
    return np.ascontiguousarray(
        wt.reshape(KC, 128, D).transpose(1, 0, 2).reshape(128, KC * D)).astype(nbf)


def _make_mask4():
    m = np.zeros((128, 512), np.float32)
    for p in range(128):
        for b4 in range(4):
            s = p // 8
            m[p, b4 * 128 + s * 8:b4 * 128 + (s + 1) * 8] = 1.0
    return m.astype(nbf)


_NC_CACHE = {}


def _get_nc(seq, nblk):
    key = (seq, nblk)
    if key not in _NC_CACHE:
        _NC_CACHE[key] = build_bass(seq, nblk)
    return _NC_CACHE[key]


def prepare_in_maps(inputs, seq=S):
    """Host-side packing: returns per-core in_maps."""
    pe = inputs["pe"].astype(np.float32)
    x_all = inputs["input_seq"].astype(np.float32) + pe  # [B, S, D]
    o_all = inputs["output_seq"].astype(np.float32)
    rs = 1.0 / math.sqrt(HD)

    weights = {}
    biases = np.zeros((128, len(LAYERS) * KC), np.float32)
    for li, (wn, bn) in enumerate(LAYERS):
        w = inputs[wn].astype(np.float32)
        b = inputs[bn].astype(np.float32)
        if wn in ("enc_wq", "dec_s_wq", "dec_c_wq"):
            w = w * rs
            b = b * rs
        weights[f"w{li}"] = _pack_w(w)
        biases[:, li * KC:(li + 1) * KC] = b.reshape(KC, 128).T

    mask4 = _make_mask4()
    ident = np.eye(128, dtype=nbf)

    in_maps = []
    for c in range(NCORES):
        in_maps.append({
            "xT": _pack_fm(x_all[c][:seq]),
            "oT": _pack_fm(o_all[c][:seq]),
            "biases": biases,
            "mask4": mask4,
            "ident": ident,
            **weights,
        })
    return in_maps


def unpack_out(y):
    """[128, KC, S] f32 -> [S, D]."""
    return np.ascontiguousarray(y.transpose(2, 1, 0).reshape(-1, D))


def kernel(**inputs):
    nc = _get_nc(S, 2)
    in_maps = prepare_in_maps(inputs, S)
    res = run_bass_kernel_spmd(nc, in_maps, core_ids=list(range(NCORES)))
    out = np.stack([unpack_out(res.results[c]["y"]) for c in range(NCORES)])
    return out.astype(np.float32)


# revision 16
# speedup vs baseline: 115.6532x; 115.6532x over previous
"""Trainium2 Bass kernel for nn_DialogueTransformer.

Reference computation (per batch element b, S=2048 positions, D=1024, H=8 heads
of HD=128):
  x = input_seq + pe
  enc:  q/k/v = x@Wq.T+bq ... ; per-position 8x8 head-mixing softmax attention
        (reshape WITHOUT transpose => heads attend within the same position);
        FFN relu(a@W1.T+b1)@W2.T+b2
  dec:  self-attn on output_seq, cross-attn (q from self-attn out, k/v from
        enc_out), FFN, final fc.

Sharding: pure data-parallel over batch (8 cores, one batch element each).
No collectives.

Device layout: all activations are FEATURE-MAJOR [1024 features, S positions],
split into 8 partition-chunks of 128. q/k/v and attention outputs live in an
INTERLEAVED feature-major layout  [128, S*8]  with column  s*8 + head. This
makes the per-position 8x8 head mixing expressible as dense 128x128 PE matmuls
over 16-position groups:
  scores:  psS[(s,i),(s',j)] = qI_g.T @ kI_g   (diag 8x8 blocks are the real
           scores; off-diag cross-position garbage is masked away)
  softmax: exp on ACT, mask-mul + row-reduce + reciprocal + per-partition
           normalize on DVE
  mix:     transpose W and vI groups on PE, then  out[h,(s,i)] = vPI.T @ WT
The 1/sqrt(HD) score scale is folded into Wq/bq on the host.

Everything is bf16 on the PE with fp32 PSUM accumulation; positions are
processed in NBLK blocks so all block activations fit SBUF; weights stream per
block.
"""

import math
import numpy as np
import ml_dtypes
from contextlib import ExitStack

import concourse.bass as bass
import concourse.tile as tile
from concourse import bacc, mybir
from concourse.bass import ds
from concourse.bass_utils import run_bass_kernel_spmd

BF16 = mybir.dt.bfloat16
F32 = mybir.dt.float32
nbf = ml_dtypes.bfloat16

B, S, D, H, HD = 8, 2048, 1024, 8, 128
NCORES = 8
KC = D // 128          # 8 feature chunks
AF = mybir.ActivationFunctionType
ALU = mybir.AluOpType

# layer index -> (weight name, bias name)
LAYERS = [
    ("enc_wq", "enc_bq"), ("enc_wk", "enc_bk"), ("enc_wv", "enc_bv"),
    ("enc_w1", "enc_b1"), ("enc_w2", "enc_b2"),
    ("dec_s_wq", "dec_s_bq"), ("dec_s_wk", "dec_s_bk"), ("dec_s_wv", "dec_s_bv"),
    ("dec_c_wq", "dec_c_bq"), ("dec_c_wk", "dec_c_bk"), ("dec_c_wv", "dec_c_bv"),
    ("dec_w1", "dec_b1"), ("dec_w2", "dec_b2"), ("fc_w", "fc_b"),
]
LIDX = {name: i for i, (name, _) in enumerate(LAYERS)}


def build_bass(seq=S, nblk=2, repeat=1, variant=""):
    """Build + compile the per-core Bass program. seq = positions per core.

    repeat > 1 wraps the whole body in a device-side For_i loop (for timing:
    marginal wall time per extra iteration = steady-state kernel time).
    variant: timing-only ablations — "noattn" stubs attention with a copy,
    "nostride" replaces strided interleaved rhs reads with contiguous ones
    (results wrong; timing only)."""
    sb_pos = seq // nblk          # positions per block
    nn = sb_pos // 512            # 512-wide position chunks per block
    nch = sb_pos // 64            # attention 4-group chunks per block
    assert sb_pos % 512 == 0 and sb_pos % 64 == 0

    nc = bacc.Bacc("TRN2", target_bir_lowering=False, debug=False)

    xT_d = nc.dram_tensor("xT", [128, KC, seq], BF16, kind="ExternalInput")
    oT_d = nc.dram_tensor("oT", [128, KC, seq], BF16, kind="ExternalInput")
    w_d = [
        nc.dram_tensor(f"w{i}", [128, KC * D], BF16, kind="ExternalInput")
        for i in range(len(LAYERS))
    ]
    bias_d = nc.dram_tensor("biases", [128, len(LAYERS) * KC], F32,
                            kind="ExternalInput")
    mask_d = nc.dram_tensor("mask4", [128, 512], BF16, kind="ExternalInput")
    id_d = nc.dram_tensor("ident", [128, 128], BF16, kind="ExternalInput")
    y_d = nc.dram_tensor("y", [128, KC, seq], F32, kind="ExternalOutput")

    with tile.TileContext(nc) as tc:
        with ExitStack() as ctx:
            const = ctx.enter_context(tc.tile_pool(name="const", bufs=1))
            big = ctx.enter_context(tc.tile_pool(name="big", bufs=1))
            wpool = ctx.enter_context(tc.tile_pool(name="wpool", bufs=2))
            sm = ctx.enter_context(tc.tile_pool(name="sm", bufs=3))
            stg = ctx.enter_context(tc.tile_pool(name="stg", bufs=3))
            ps = ctx.enter_context(tc.tile_pool(name="ps", bufs=4, space="PSUM"))
            pst = ctx.enter_context(tc.tile_pool(name="pst", bufs=2, space="PSUM"))

            bsb = const.tile([128, len(LAYERS) * KC], F32, tag="bias")
            nc.sync.dma_start(bsb[:], bias_d.ap())
            msb = const.tile([128, 512], BF16, tag="mask")
            nc.sync.dma_start(msb[:], mask_d.ap())
            isb = const.tile([128, 128], BF16, tag="ident")
            nc.sync.dma_start(isb[:], id_d.ap())

            def load_weight(li):
                w = wpool.tile([128, KC * D], BF16, tag="w", name=f"w_{li}")
                nc.sync.dma_start(w[:], w_d[li].ap())
                return w

            def load_act(dram, blk, tag):
                t = big.tile([128, KC * sb_pos], BF16, tag=tag, name=tag)
                nc.sync.dma_start(
                    t.rearrange("p (k s) -> p k s", k=KC),
                    dram.ap()[:, :, ds(blk * sb_pos, sb_pos)],
                )
                return t

            def proj(li, rhs_fn, evac_fn):
                """y_chunk[m] = W_li[m,:] @ rhs ; evac_fn(ps, m, n)."""
                w = load_weight(li)
                for m in range(KC):
                    for n in range(nn):
                        pt = ps.tile([128, 512], F32, tag="ps", name=f"ps_{li}_{m}_{n}")
                        for k in range(KC):
                            nc.tensor.matmul(
                                pt[:],
                                w[:, ds(k * D + m * 128, 128)],
                                rhs_fn(k, n),
                                start=(k == 0), stop=(k == KC - 1),
                            )
                        evac_fn(pt, m, n)

            def chunk_rhs(src):
                """rhs from a feature-chunk-major [128, KC*sb_pos] buffer."""
                return lambda k, n: src[:, ds(k * sb_pos + n * 512, 512)]

            def inter_rhs(src_r):
                """rhs from an interleaved [128, sb_pos, 8] buffer."""
                if variant == "nostride":
                    return lambda k, n: src_r.tensor.ap()[:, ds(n * 512, 512)]
                return lambda k, n: src_r[:, ds(n * 512, 512), k]

            def evac_interleaved(dst_r, li):
                """psum -> interleaved dst with bias; alternate ACT/DVE."""
                def f(pt, m, n):
                    bias_ap = bsb[:, ds(li * KC + m, 1)]
                    if variant == "contigevac":  # timing-only ablation
                        dst = dst_r.tensor.ap()[:, ds(n * 512, 512)]
                    else:
                        dst = dst_r[:, ds(n * 512, 512), m]
                    if m % 2 == 0:
                        nc.scalar.activation(dst, pt[:], AF.Identity, bias=bias_ap)
                    else:
                        nc.vector.tensor_scalar(dst, pt[:], bias_ap, None, ALU.add)
                return f

            def evac_chunks(dst, li, relu=False):
                def f(pt, m, n):
                    bias_ap = bsb[:, ds(li * KC + m, 1)]
                    dst_ap = dst[:, ds(m * sb_pos + n * 512, 512)]
                    if m % 2 == 0:
                        nc.scalar.activation(
                            dst_ap, pt[:], AF.Relu if relu else AF.Identity,
                            bias=bias_ap)
                    elif relu:
                        nc.vector.tensor_scalar(
                            dst_ap, pt[:], bias_ap, 0.0, ALU.add, ALU.max)
                    else:
                        nc.vector.tensor_scalar(
                            dst_ap, pt[:], bias_ap, None, ALU.add)
                return f

            def evac_out(blk, li):
                def f(pt, m, n):
                    bias_ap = bsb[:, ds(li * KC + m, 1)]
                    t = stg.tile([128, 512], F32, tag="ystg", name="ystg")
                    if m % 2 == 0:
                        nc.scalar.activation(t[:], pt[:], AF.Identity, bias=bias_ap)
                    else:
                        nc.vector.tensor_scalar(t[:], pt[:], bias_ap, None, ALU.add)
                    nc.sync.dma_start(
                        y_d.ap()[:, m, ds(blk * sb_pos + n * 512, 512)], t[:])
                return f

            def attention(qI, kI, vI, aI):
                if variant == "noattn":
                    for m in range(KC):
                        nc.vector.tensor_copy(
                            aI[:, ds(m * sb_pos, sb_pos)],
                            qI[:, ds(m * sb_pos, sb_pos)])
                    return

                # 3-stage software pipeline over 64-position chunks so the
                # ACT/DVE softmax chain of chunk ch runs under the PE work of
                # chunks ch+1/ch+2 (PE executes its queue in order; without
                # the skew the W-transpose would head-of-line-block PE).
                def s0(ch):
                    base = ch * 512
                    psS = ps.tile([128, 512], F32, tag="psS", name="psS",
                                  bufs=2)
                    for g in range(4):
                        sl = ds(base + g * 128, 128)
                        nc.tensor.matmul(psS[:, ds(g * 128, 128)],
                                         qI[:, sl], kI[:, sl],
                                         start=True, stop=True)
                    return psS

                def s1(ch, psS):
                    E = sm.tile([128, 512], BF16, tag="E", name="E")
                    nc.scalar.activation(E[:], psS[:], AF.Exp)
                    Em = sm.tile([128, 512], BF16, tag="Em", name="Em")
                    nc.vector.tensor_mul(Em[:], E[:], msb[:])
                    den = sm.tile([128, 4], F32, tag="den", name="den")
                    nc.vector.reduce_sum(
                        den[:], Em.rearrange("p (g c) -> p g c", g=4),
                        axis=mybir.AxisListType.X)
                    R = sm.tile([128, 4], F32, tag="R", name="R")
                    nc.vector.reciprocal(R[:], den[:])
                    Wn = sm.tile([128, 512], BF16, tag="Wn", name="Wn")
                    for g in range(4):
                        nc.vector.tensor_scalar_mul(
                            Wn[:, ds(g * 128, 128)], Em[:, ds(g * 128, 128)],
                            R[:, ds(g, 1)])
                    return Wn

                def s2(ch, Wn):
                    base = ch * 512
                    ptv = pst.tile([128, 512], BF16, tag="pst", name="ptv")
                    for g in range(4):
                        nc.tensor.transpose(ptv[:, ds(g * 128, 128)],
                                            vI[:, ds(base + g * 128, 128)], isb[:])
                    vP = sm.tile([128, 512], BF16, tag="vP", name="vP")
                    nc.scalar.copy(vP[:], ptv[:])
                    ptw = pst.tile([128, 512], BF16, tag="pst", name="ptw")
                    for g in range(4):
                        nc.tensor.transpose(ptw[:, ds(g * 128, 128)],
                                            Wn[:, ds(g * 128, 128)], isb[:])
                    WT = sm.tile([128, 512], BF16, tag="WT", name="WT")
                    nc.vector.tensor_copy(WT[:], ptw[:])
                    psO = ps.tile([128, 512], F32, tag="ps", name="psO")
                    for g in range(4):
                        sl = ds(g * 128, 128)
                        nc.tensor.matmul(psO[:, sl], vP[:, sl], WT[:, sl],
                                         start=True, stop=True)
                    nc.scalar.copy(aI[:, ds(base, 512)], psO[:])

                state = {}
                for ch in range(nch + 2):
                    if ch < nch:
                        state[ch] = s0(ch)
                    if 1 <= ch < nch + 1:
                        state[ch - 1] = s1(ch - 1, state[ch - 1])
                    if ch >= 2:
                        s2(ch - 2, state.pop(ch - 2))

            def deinter(aI_r, tag, nm):
                """aI interleaved -> chunk-major copy (strided reads on
                DVE/ACT are ~1x; strided PE rhs reads were ~4x slower)."""
                a_cm = big.tile([128, KC * sb_pos], BF16, tag=tag, name=nm)
                for k in range(KC):
                    dst = a_cm[:, ds(k * sb_pos, sb_pos)]
                    src = aI_r[:, ds(0, sb_pos), k]
                    if k % 2 == 0:
                        nc.scalar.copy(dst, src)
                    else:
                        nc.vector.tensor_copy(dst, src)
                return a_cm

            def qkv_layers(src, li_q, li_k, li_v):
                qI = big.tile([128, sb_pos * 8], BF16, tag="qI", name="qI")
                kI = big.tile([128, sb_pos * 8], BF16, tag="kI", name="kI")
                vI = big.tile([128, sb_pos * 8], BF16, tag="vI", name="vI")
                qI_r = qI.rearrange("p (s e) -> p s e", e=8)
                kI_r = kI.rearrange("p (s e) -> p s e", e=8)
                vI_r = vI.rearrange("p (s e) -> p s e", e=8)
                rhs = chunk_rhs(src)
                proj(li_q, rhs, evac_interleaved(qI_r, li_q))
                proj(li_k, rhs, evac_interleaved(kI_r, li_k))
                proj(li_v, rhs, evac_interleaved(vI_r, li_v))
                return qI, kI, vI

            def body():
                for blk in range(nblk):
                    whole_block(blk)

            def whole_block(blk):
                # ---------------- encoder ----------------
                xc = load_act(xT_d, blk, "xc")
                qI, kI, vI = qkv_layers(
                    xc, LIDX["enc_wq"], LIDX["enc_wk"], LIDX["enc_wv"])
                aI = big.tile([128, sb_pos * 8], BF16, tag="aI", name="aI_enc")
                attention(qI, kI, vI, aI)
                aI_r = aI.rearrange("p (s e) -> p s e", e=8)
                a_cm = deinter(aI_r, "xc", "acm_enc")
                h1 = big.tile([128, KC * sb_pos], BF16, tag="h1", name="h1_enc")
                proj(LIDX["enc_w1"], chunk_rhs(a_cm),
                     evac_chunks(h1, LIDX["enc_w1"], relu=True))
                eo = big.tile([128, KC * sb_pos], BF16, tag="eo", name="eo")
                proj(LIDX["enc_w2"], chunk_rhs(h1),
                     evac_chunks(eo, LIDX["enc_w2"]))

                # ---------------- decoder self-attention ----------------
                oc = load_act(oT_d, blk, "oc")
                qI, kI, vI = qkv_layers(
                    oc, LIDX["dec_s_wq"], LIDX["dec_s_wk"], LIDX["dec_s_wv"])
                aI = big.tile([128, sb_pos * 8], BF16, tag="aI", name="aI_sa")
                attention(qI, kI, vI, aI)
                aI_r = aI.rearrange("p (s e) -> p s e", e=8)
                a_cm = deinter(aI_r, "h1", "acm_sa")

                # ---------------- decoder cross-attention ----------------
                qI = big.tile([128, sb_pos * 8], BF16, tag="qI", name="qI_c")
                qI_r = qI.rearrange("p (s e) -> p s e", e=8)
                proj(LIDX["dec_c_wq"], chunk_rhs(a_cm),
                     evac_interleaved(qI_r, LIDX["dec_c_wq"]))
                kI = big.tile([128, sb_pos * 8], BF16, tag="kI", name="kI_c")
                kI_r = kI.rearrange("p (s e) -> p s e", e=8)
                proj(LIDX["dec_c_wk"], chunk_rhs(eo),
                     evac_interleaved(kI_r, LIDX["dec_c_wk"]))
                vI = big.tile([128, sb_pos * 8], BF16, tag="vI", name="vI_c")
                vI_r = vI.rearrange("p (s e) -> p s e", e=8)
                proj(LIDX["dec_c_wv"], chunk_rhs(eo),
                     evac_interleaved(vI_r, LIDX["dec_c_wv"]))
                aI = big.tile([128, sb_pos * 8], BF16, tag="aI", name="aI_ca")
                attention(qI, kI, vI, aI)
                aI_r = aI.rearrange("p (s e) -> p s e", e=8)
                a_cm = deinter(aI_r, "qI", "acm_ca")

                # ---------------- decoder FFN + fc ----------------
                h1 = big.tile([128, KC * sb_pos], BF16, tag="h1", name="h1_dec")
                proj(LIDX["dec_w1"], chunk_rhs(a_cm),
                     evac_chunks(h1, LIDX["dec_w1"], relu=True))
                d1 = big.tile([128, KC * sb_pos], BF16, tag="xc", name="d1")
                proj(LIDX["dec_w2"], chunk_rhs(h1),
                     evac_chunks(d1, LIDX["dec_w2"]))
                proj(LIDX["fc_w"], chunk_rhs(d1), evac_out(blk, LIDX["fc_w"]))

            if repeat > 1:
                with tc.For_i(0, repeat, 1):
                    body()
            else:
                body()

    nc.compile()
    return nc


def _pack_fm(a):
    """[S, D] fp32 -> feature-major packed [128, KC, S] bf16."""
    s = a.shape[0]
    return np.ascontiguousarray(
        a.T.reshape(KC, 128, s).transpose(1, 0, 2)).astype(nbf)


def _pack_w(w):
    """[d_out, d_in] -> lhsT packed [128, KC*D] bf16 (chunk kc of d_in)."""
    wt = np.ascontiguousarray(w.T)  # [d_in, d_out]
    return np.ascontiguousarray(
        wt.reshape(KC, 128, D).transpose(1, 0, 2).reshape(128, KC * D)).astype(nbf)


def _make_mask4():
    m = np.zeros((128, 512), np.float32)
    for p in range(128):
        for b4 in range(4):
            s = p // 8
            m[p, b4 * 128 + s * 8:b4 * 128 + (s + 1) * 8] = 1.0
    return m.astype(nbf)


_NC_CACHE = {}


def _get_nc(seq, nblk):
    key = (seq, nblk)
    if key not in _NC_CACHE:
        _NC_CACHE[key] = build_bass(seq, nblk)
    return _NC_CACHE[key]


def prepare_in_maps(inputs, seq=S):
    """Host-side packing: returns per-core in_maps."""
    pe = inputs["pe"].astype(np.float32)
    x_all = inputs["input_seq"].astype(np.float32) + pe  # [B, S, D]
    o_all = inputs["output_seq"].astype(np.float32)
    rs = 1.0 / math.sqrt(HD)

    weights = {}
    biases = np.zeros((128, len(LAYERS) * KC), np.float32)
    for li, (wn, bn) in enumerate(LAYERS):
        w = inputs[wn].astype(np.float32)
        b = inputs[bn].astype(np.float32)
        if wn in ("enc_wq", "dec_s_wq", "dec_c_wq"):
            w = w * rs
            b = b * rs
        weights[f"w{li}"] = _pack_w(w)
        biases[:, li * KC:(li + 1) * KC] = b.reshape(KC, 128).T

    mask4 = _make_mask4()
    ident = np.eye(128, dtype=nbf)

    in_maps = []
    for c in range(NCORES):
        in_maps.append({
            "xT": _pack_fm(x_all[c][:seq]),
            "oT": _pack_fm(o_all[c][:seq]),
            "biases": biases,
            "mask4": mask4,
            "ident": ident,
            **weights,
        })
    return in_maps


def unpack_out(y):
    """[128, KC, S] f32 -> [S, D]."""
    return np.ascontiguousarray(y.transpose(2, 1, 0).reshape(-1, D))


def kernel(**inputs):
    nc = _get_nc(S, 2)
    in_maps = prepare_in_maps(inputs, S)
    res = run_bass_kernel_spmd(nc, in_maps, core_ids=list(range(NCORES)))
    out = np.stack([unpack_out(res.results[c]["y"]) for c in range(NCORES)])
    return out.astype(np.float32)
